# revision 1
# baseline (speedup 1.0000x reference)
"""DiscreteContinuousConvS2 Trainium2 kernel (8-core SPMD).

Math (reference):
    y[c, (k,h), p] = sum_j psi_vals[n] * x[c, lat[n], (lon[n]+p) % L]
        over nonzeros n of COO row (k, h)
    out[o, h, p]  = sum_{c,k} weight[o,c,k] * y[c,(k,h),p] + bias[o]

Kernel formulation:
    out[o, h, p] = sum_c sum_j w[o, c, k_j] * (val_j * x[c, lat_j, lon_j+p])

val is folded into the MOVING matmul operand (per-partition scale of the
gathered tile), so the stationary operand wj[s, (c,o)] is shared by all
output latitudes. The 16 per-core latitudes are processed in two groups of
8: all 8 gathers land in one group tile, one broadcast multiply applies
val, and then for each input channel c the group's 4 width-512 column
chunks are PSUM-accumulated with a single stationary load (redundant
LDWEIGHTS are deleted by a peephole pass — the PE array retains stationary
weights across matmuls). Each group uses one 4-bank PSUM tile, one fused
bias-add/PSUM-drain, and one output DMA.

Gather: x2 = fp16(x) stored channel-minor and doubled along longitude
([lat, 2L, c] in DRAM), so nonzero j's full (256 lon x 32 c) window is one
contiguous 8192-element span starting at element (lat_j*2L + lon_j)*C.
One indirect-DMA instruction per latitude gathers all its j-windows.

Sharding: output latitudes split 16-per-core across 8 cores; psi tables are
repacked per-core on the host (index arithmetic + value reordering only —
all floating-point arithmetic runs on device).
"""

import numpy as np

import bass_rust
import concourse.bass as bass
import concourse.mybir as mybir
import concourse.tile as tile
from concourse import bass_utils
from concourse._compat import axon_active

N_CORES = 8
B, C, H, L = 1, 32, 128, 256
K = 3
H_LOC = H // N_CORES
L2 = 2 * L
X2_N = H * L2 * C
G8 = 8                     # latitudes per matmul group

F32 = mybir.dt.float32
F16 = mybir.dt.float16
I32 = mybir.dt.int32

TC = tile.TileContext


def split_multiwaits(nc):
    """Split instructions carrying >1 sync wait into single-wait NoOps.

    This walrus build rejects any instruction with more than one sync wait
    and any sync wait at all on InstISA. A wait executes at the engine's
    sequencer before the instruction issues, so hoisting waits onto NoOps
    immediately preceding the instruction (same engine, same block
    position) is semantics-preserving.
    """
    ctr = 0
    for f in nc.m.functions:
        for b in f.blocks:
            out = []
            changed = False
            for inst in b.instructions:
                si = inst.sync_info
                nkeep = 0 if isinstance(inst, mybir.InstISA) else 1
                if si is not None and len(si.on_wait) > nkeep:
                    changed = True
                    waits = list(si.on_wait)
                    nhoist = len(waits) - nkeep
                    for w in waits[:nhoist]:
                        n = mybir.InstNoOp(name=f"mwsplit_{ctr}", ins=[],
                                           outs=[])
                        ctr += 1
                        n.engine = inst.engine
                        n.sync_info = bass_rust.SyncInfo(on_wait=[w],
                                                         on_update=[])
                        out.append(n)
                    inst.sync_info = bass_rust.SyncInfo(
                        on_wait=list(waits[nhoist:]),
                        on_update=list(si.on_update))
                out.append(inst)
            if changed:
                b.instructions = out
    return nc


def dedupe_ldweights(nc):
    """Drop InstLdweights that reload the identical stationary operand.

    The PE array retains stationary weights across InstMatmult, so a
    Ldweights whose access pattern matches the previous one (with only
    PE Matmult/NoOp in between) is redundant. Only sync-free duplicates
    are removed.
    """
    for f in nc.m.functions:
        for b in f.blocks:
            out = []
            last_ld = None
            for inst in b.instructions:
                if isinstance(inst, mybir.InstLdweights):
                    si = inst.sync_info
                    clean = si is None or (not si.on_wait and
                                           not si.on_update)
                    key = (str(inst.ins[0]), str(inst.perf_mode),
                           str(inst.is_transpose), str(inst.tile_position))
                    if clean and last_ld == key:
                        continue
                    last_ld = key if clean else None
                elif (inst.engine == mybir.EngineType.PE
                      and not isinstance(inst, (mybir.InstMatmult,
                                                mybir.InstNoOp))):
                    last_ld = None
                out.append(inst)
            b.instructions = out
    return nc


def host_prep(x, psi_vals, weight, bias, psi_ik, psi_lat_out, psi_lat_in,
              psi_lon_in):
    """Repack COO tables into per-core gather-offset / value / weight arrays.

    Only index arithmetic and array reshuffles happen here; every
    floating-point operation (cast, scale, matmul, bias) runs on device.
    Returns (in_maps, pj) where pj = padded j-slots per output latitude.
    """
    nnz = psi_vals.shape[0]
    seg = psi_ik.astype(np.int64) * H + psi_lat_out.astype(np.int64)
    order = np.argsort(seg, kind="stable")
    seg_s = seg[order]
    counts = np.bincount(seg_s, minlength=K * H)
    maxc = int(counts.max())
    pj = K * maxc
    assert pj <= 128, f"padded nonzeros per latitude {pj} > 128 partitions"

    row_lat = np.zeros((K * H, maxc), np.int64)
    row_lon = np.zeros((K * H, maxc), np.int64)
    row_val = np.zeros((K * H, maxc), np.float32)
    starts = np.zeros(K * H + 1, np.int64)
    np.cumsum(counts, out=starts[1:])
    col = np.arange(nnz) - starts[seg_s]
    row_lat[seg_s, col] = psi_lat_in[order]
    row_lon[seg_s, col] = psi_lon_in[order]
    row_val[seg_s, col] = psi_vals[order]

    k_of_s = np.arange(pj) // maxc

    wmat = np.asarray(weight, np.float32)   # [O=32, C=32, K]
    wj = np.transpose(wmat[:, :, k_of_s], (2, 1, 0))        # [pj, C, O=32]
    wj = np.ascontiguousarray(wj.reshape(pj, C * 32))

    xt = np.ascontiguousarray(
        np.transpose(np.asarray(x, np.float32).reshape(C, H, L), (1, 2, 0)))
    bias_np = np.ascontiguousarray(np.asarray(bias, np.float32).reshape(32, 1))

    in_maps = []
    for m in range(N_CORES):
        hs = m * H_LOC
        idx = np.zeros((H_LOC, pj), np.int32)
        val = np.zeros((H_LOC, pj), np.float32)
        for hl in range(H_LOC):
            r = k_of_s * H + (hs + hl)
            jw = np.arange(pj) % maxc
            lat = row_lat[r, jw]
            lon = row_lon[r, jw]
            idx[hl] = ((lat * L2 + lon) * C).astype(np.int32)
            val[hl] = row_val[r, jw]
        in_maps.append({
            "xt": xt,
            "idxT": np.ascontiguousarray(idx.T),
            "valT": np.ascontiguousarray(val.T),
            "wj": wj,
            "bias": bias_np,
        })
    return in_maps, pj


def build_program(pj, repeat=1, split=True):
    nc = bass.Bass("TRN2", target_bir_lowering=False, debug=not axon_active(),
                   num_devices=N_CORES)

    xt_d = nc.dram_tensor("xt", [H, L, C], F32, kind="ExternalInput")
    idxT_d = nc.dram_tensor("idxT", [pj, H_LOC], I32, kind="ExternalInput")
    valT_d = nc.dram_tensor("valT", [pj, H_LOC], F32, kind="ExternalInput")
    wj_d = nc.dram_tensor("wj", [pj, C * 32], F32, kind="ExternalInput")
    bias_d = nc.dram_tensor("bias", [32, 1], F32, kind="ExternalInput")
    out_d = nc.dram_tensor("out", [32, H_LOC, L], F32, kind="ExternalOutput")

    LGRP = 32
    with TC(nc, num_cores=N_CORES) as tc:
        with (
            tc.tile_pool(name="dram", bufs=1, space="DRAM") as dpool,
            tc.tile_pool(name="const", bufs=1) as cpool,
        ):
            # --- x2 = fp16(xt) doubled along longitude: [lat, 2L, c] ---
            x2 = dpool.tile([X2_N, 1], F16)
            x2v = x2[:].rearrange("(h s c) one -> h (s c one)", h=H, s=L2)
            xtv = xt_d.ap().rearrange("h l c -> h (l c)")
            with tc.tile_pool(name="prep", bufs=2) as prpool:
                for lg in range(0, H, LGRP):
                    xf = prpool.tile([LGRP, L * C], F32, tag="xf")
                    nc.sync.dma_start(xf[:], xtv[lg:lg + LGRP])
                    xh = prpool.tile([LGRP, L * C], F16, tag="xh")
                    nc.vector.tensor_copy(xh[:], xf[:])
                    nc.sync.dma_start(x2v[lg:lg + LGRP, 0:L * C], xh[:])
                    nc.sync.dma_start(x2v[lg:lg + LGRP, L * C:L2 * C], xh[:])

            # --- constants (loop-invariant, loaded once) ---
            wj_sb = cpool.tile([pj, C * 32], F32)
            nc.sync.dma_start(wj_sb[:], wj_d.ap())
            wj16 = cpool.tile([pj, C * 32], F16)
            nc.vector.tensor_copy(wj16[:], wj_sb[:])
            bias_sb = cpool.tile([32, 1], F32)
            nc.sync.dma_start(bias_sb[:], bias_d.ap())
            idx_all = cpool.tile([pj, H_LOC], I32)
            nc.sync.dma_start(idx_all[:], idxT_d.ap())
            val_all = cpool.tile([pj, H_LOC], F32)
            nc.sync.dma_start(val_all[:], valT_d.ap())

            with (
                tc.tile_pool(name="g2", bufs=1) as g2pool,
                tc.tile_pool(name="work", bufs=3) as wpool,
                tc.tile_pool(name="psum", bufs=2, space="PSUM") as ppool,
            ):
                for it in range(repeat):
                    for grp in range(H_LOC // G8):
                        g2 = g2pool.tile([pj, G8 * L * C], F16, tag="g2",
                                         name=f"g2_{it}_{grp}")
                        for l in range(G8):
                            hl = grp * G8 + l
                            gsl = g2[:, l * L * C:(l + 1) * L * C]
                            nc.gpsimd.indirect_dma_start(
                                out=gsl, out_offset=None, in_=x2[:, :],
                                in_offset=bass.IndirectOffsetOnAxis(
                                    ap=idx_all[:, hl:hl + 1], axis=0))

                        # g2 *= val (broadcast over lon and channel); split
                        # in two — a merged free dim of 65536 overflows the
                        # 16-bit num_elem ISA field.
                        g2v = g2[:].rearrange("s (h l p c) -> s h l p c",
                                              h=2, l=G8 // 2, c=C)
                        valv = val_all[:, grp * G8:(grp + 1) * G8].rearrange(
                            "s (h l u v) -> s h l u v", h=2, u=1, v=1)
                        for hh in range(2):
                            nc.vector.tensor_mul(
                                g2v[:, hh], g2v[:, hh],
                                valv[:, hh].broadcast_to(
                                    [pj, G8 // 2, L, C]))

                        # per channel: 4 width-512 chunks into one 4-bank
                        # PSUM tile (each chunk stays inside one bank),
                        # PSUM-accumulated over c with a shared stationary
                        psum = ppool.tile([32, 4 * 512], F32, tag="ps",
                                          name=f"ps_{it}_{grp}")
                        g2m = g2[:].rearrange(
                            "s (ch l p c) -> s c ch (l p)", ch=4, l=2, c=C)
                        for c in range(C):
                            for ch in range(4):
                                nc.tensor.matmul(
                                    psum[:, ch * 512:(ch + 1) * 512],
                                    wj16[:, c * 32:(c + 1) * 32],
                                    g2m[:, c, ch], start=(c == 0),
                                    stop=(c == C - 1))

                        out_sb = wpool.tile([32, 4 * 512], F32, tag="osb",
                                            name=f"osb_{it}_{grp}")
                        nc.vector.tensor_scalar_add(out_sb[:], psum[:],
                                                    bias_sb[:])
                        nc.sync.dma_start(
                            out_d.ap()[:, grp * G8:(grp + 1) * G8, :],
                            out_sb[:])
    dedupe_ldweights(nc)
    if split:
        split_multiwaits(nc)
    return nc


_PROGRAM_CACHE = {}


def _get_program(pj, repeat=1):
    key = (pj, repeat)
    if key not in _PROGRAM_CACHE:
        _PROGRAM_CACHE[key] = build_program(pj, repeat)
    return _PROGRAM_CACHE[key]


def kernel(x, psi_vals, weight, bias, psi_ik, psi_lat_out, psi_lat_in,
           psi_lon_in, nlat_out, nlon_out):
    x = np.asarray(x)
    assert x.shape == (B, C, H, L), x.shape
    assert int(nlat_out) == H and int(nlon_out) == L

    in_maps, pj = host_prep(
        x, np.asarray(psi_vals), np.asarray(weight), np.asarray(bias),
        np.asarray(psi_ik), np.asarray(psi_lat_out),
        np.asarray(psi_lat_in), np.asarray(psi_lon_in))

    nc = _get_program(pj)
    res = bass_utils.run_bass_kernel_spmd(nc, in_maps,
                                          core_ids=list(range(N_CORES)))
    out = np.concatenate([res.results[m]["out"] for m in range(N_CORES)],
                         axis=1)
    return out.reshape(B, 32, H, L).astype(np.float32)



# revision 6
# speedup vs baseline: 131.0164x; 131.0164x over previous
"""DiscreteContinuousConvS2 Trainium2 kernel (8-core SPMD), v3.

Math (reference):
    y[c, (k,h), p] = sum_j psi_vals[n] * x[c, lat[n], (lon[n]+p) % L]
        over nonzeros n of COO row (k, h)
    out[o, h, p]  = sum_{c,k} weight[o,c,k] * y[c,(k,h),p] + bias[o]

Kernel formulation (per output latitude h):
    out[o, h, p] = sum_c sum_j wstat[j, o | c, h] * g[j, p, c]
    wstat[j, o | c, h] = weight[o, c, k(j)] * val[j, h]
    g[j, p, c]        = x[c, lat_j, lon_j + p]   (gathered window)

val is folded into per-(c,h) stationary operands precomputed ON DEVICE
once outside the loop (Ldweights are free); the inner loop is a pure
PSUM chain of 32 matmuls [96,32]^T x [96,256] per latitude, where the
moving operand is a stride-32 channel slice of the gathered window.

Gather (hardware semantics: ONE offset per partition, the out row's
full free span is fetched contiguously from it): x2 = fp16(x) stored
[lat, 2L, c] (channel-minor, doubled along longitude) in DRAM, so
nonzero j's full (256 lon x 32 c) window is one contiguous 8192-element
span starting at element (lat_j*2L + lon_j)*C. One indirect-DMA per
latitude gathers its 96 windows (96 partitions x 16 KiB). The DRAM side
is presented as a flat [1, N] AP (axis=1, coef=1) so DMA descriptors
are sized by the large contiguous run, not a 1-element inner dim.

Sharding: output latitudes split 16-per-core across 8 cores; psi tables
are repacked per-core on the host (index arithmetic + value
reordering/replication only - all floating-point arithmetic runs on
device).
"""

import numpy as np

import bass_rust
import concourse.bass as bass
import concourse.mybir as mybir
import concourse.tile as tile
from concourse import bass_utils
from concourse._compat import axon_active

N_CORES = 8
B, C, H, L = 1, 32, 128, 256
K = 3
H_LOC = H // N_CORES
L2 = 2 * L
X2_N = H * L2 * C

F32 = mybir.dt.float32
F16 = mybir.dt.float16
I32 = mybir.dt.int32

TC = tile.TileContext


def split_multiwaits(nc):
    """Split instructions carrying >1 sync wait into single-wait NoOps.

    This walrus build rejects any instruction with more than one sync wait
    and any sync wait at all on InstISA. A wait executes at the engine's
    sequencer before the instruction issues, so hoisting waits onto NoOps
    immediately preceding the instruction (same engine, same block
    position) is semantics-preserving.
    """
    ctr = 0
    for f in nc.m.functions:
        for b in f.blocks:
            out = []
            changed = False
            for inst in b.instructions:
                si = inst.sync_info
                nkeep = 0 if isinstance(inst, mybir.InstISA) else 1
                if si is not None and len(si.on_wait) > nkeep:
                    changed = True
                    waits = list(si.on_wait)
                    nhoist = len(waits) - nkeep
                    for w in waits[:nhoist]:
                        n = mybir.InstNoOp(name=f"mwsplit_{ctr}", ins=[],
                                           outs=[])
                        ctr += 1
                        n.engine = inst.engine
                        n.sync_info = bass_rust.SyncInfo(on_wait=[w],
                                                         on_update=[])
                        out.append(n)
                    inst.sync_info = bass_rust.SyncInfo(
                        on_wait=list(waits[nhoist:]),
                        on_update=list(si.on_update))
                out.append(inst)
            if changed:
                b.instructions = out
    return nc


def dedupe_ldweights(nc):
    """Drop InstLdweights that reload the identical stationary operand."""
    for f in nc.m.functions:
        for b in f.blocks:
            out = []
            last_ld = None
            for inst in b.instructions:
                if isinstance(inst, mybir.InstLdweights):
                    si = inst.sync_info
                    clean = si is None or (not si.on_wait and
                                           not si.on_update)
                    key = (str(inst.ins[0]), str(inst.perf_mode),
                           str(inst.is_transpose), str(inst.tile_position))
                    if clean and last_ld == key:
                        continue
                    last_ld = key if clean else None
                elif (inst.engine == mybir.EngineType.PE
                      and not isinstance(inst, (mybir.InstMatmult,
                                                mybir.InstNoOp))):
                    last_ld = None
                out.append(inst)
            b.instructions = out
    return nc


def host_prep(x, psi_vals, weight, bias, psi_ik, psi_lat_out, psi_lat_in,
              psi_lon_in):
    """Repack COO tables into per-core gather-offset / value / weight arrays.

    Only index arithmetic and array reshuffles/replication happen here;
    every floating-point operation (cast, w*val, matmul, bias) runs on
    device. Returns (in_maps, pj) with pj = padded j-slots per latitude.
    """
    nnz = psi_vals.shape[0]
    seg = psi_ik.astype(np.int64) * H + psi_lat_out.astype(np.int64)
    order = np.argsort(seg, kind="stable")
    seg_s = seg[order]
    counts = np.bincount(seg_s, minlength=K * H)
    maxc = int(counts.max())
    pj = K * maxc
    assert pj <= 128, f"padded nonzeros per latitude {pj} > 128 partitions"

    row_lat = np.zeros((K * H, maxc), np.int64)
    row_lon = np.zeros((K * H, maxc), np.int64)
    row_val = np.zeros((K * H, maxc), np.float32)
    starts = np.zeros(K * H + 1, np.int64)
    np.cumsum(counts, out=starts[1:])
    col = np.arange(nnz) - starts[seg_s]
    row_lat[seg_s, col] = psi_lat_in[order]
    row_lon[seg_s, col] = psi_lon_in[order]
    row_val[seg_s, col] = psi_vals[order]

    k_of_s = np.arange(pj) // maxc
    jw = np.arange(pj) % maxc

    wmat = np.asarray(weight, np.float32)   # [O=32, C=32, K]
    # wj[j, (c,o)] = w[o, c, k(j)]
    wj = np.transpose(wmat[:, :, k_of_s], (2, 1, 0))        # [pj, C, O]
    wj = np.ascontiguousarray(wj.reshape(pj, C * 32))

    xt = np.ascontiguousarray(
        np.transpose(np.asarray(x, np.float32).reshape(C, H, L), (1, 2, 0)))
    bias_np = np.ascontiguousarray(np.asarray(bias, np.float32).reshape(32, 1))

    in_maps = []
    for m in range(N_CORES):
        hs = m * H_LOC
        r = k_of_s[:, None] * H + (hs + np.arange(H_LOC))[None, :]
        lat = row_lat[r, jw[:, None]]                  # [pj, H_LOC]
        lon = row_lon[r, jw[:, None]]
        val = row_val[r, jw[:, None]].astype(np.float32)
        idx = ((lat * L2 + lon) * C).astype(np.int32)  # [pj, H_LOC]
        in_maps.append({
            "xt": xt,
            "idxT": np.ascontiguousarray(idx),
            "valT": np.ascontiguousarray(val),
            "wj": wj,
            "bias": bias_np,
        })
    return in_maps, pj


def build_program(pj, repeat=1, split=True, unroll=1, gbufs=4):
    nc = bass.Bass("TRN2", target_bir_lowering=False, debug=not axon_active(),
                   num_devices=N_CORES)

    xt_d = nc.dram_tensor("xt", [H, L, C], F32, kind="ExternalInput")
    idx_d = nc.dram_tensor("idxT", [pj, H_LOC], I32, kind="ExternalInput")
    val_d = nc.dram_tensor("valT", [pj, H_LOC], F32, kind="ExternalInput")
    wj_d = nc.dram_tensor("wj", [pj, C * 32], F32, kind="ExternalInput")
    bias_d = nc.dram_tensor("bias", [32, 1], F32, kind="ExternalInput")
    out_d = nc.dram_tensor("out", [32, H_LOC, L], F32, kind="ExternalOutput")

    LGRP = 32
    with TC(nc, num_cores=N_CORES) as tc:
        with (
            tc.tile_pool(name="dram", bufs=1, space="DRAM") as dpool,
            tc.tile_pool(name="const", bufs=1) as cpool,
        ):
            # --- x2 = fp16(xt) doubled along longitude: [lat, 2L, c] ---
            x2 = dpool.tile([X2_N, 1], F16)
            x2flat = x2[:].rearrange("(a n) one -> a (n one)", a=1)
            x2v = x2[:].rearrange("(h s c) one -> h (s c one)", h=H, s=L2)
            with tc.tile_pool(name="prep", bufs=2) as prpool:
                for lg in range(0, H, LGRP):
                    xf = prpool.tile([LGRP, L * C], F32, tag="xf")
                    nc.sync.dma_start(xf[:], xt_d.ap().rearrange(
                        "h l c -> h (l c)")[lg:lg + LGRP])
                    xh = prpool.tile([LGRP, L * C], F16, tag="xh")
                    nc.vector.tensor_copy(xh[:], xf[:])
                    nc.sync.dma_start(x2v[lg:lg + LGRP, 0:L * C], xh[:])
                    nc.sync.dma_start(x2v[lg:lg + LGRP, L * C:L2 * C], xh[:])

            # --- constants (loop-invariant, loaded once) ---
            idx_sb = cpool.tile([pj, H_LOC], I32)
            nc.sync.dma_start(idx_sb[:], idx_d.ap())
            bias_sb = cpool.tile([32, 1], F32)
            nc.sync.dma_start(bias_sb[:], bias_d.ap())

            # wstat[j, (c,h,o)] = wj[j,(c,o)] * val[j,h], cast to fp16
            wstat = cpool.tile([pj, C * H_LOC * 32], F16)
            with tc.tile_pool(name="wprep", bufs=1) as wpool_:
                wj_sb = wpool_.tile([pj, C * 32], F32, tag="wj")
                nc.sync.dma_start(wj_sb[:], wj_d.ap())
                val_sb = wpool_.tile([pj, H_LOC], F32, tag="val")
                nc.sync.dma_start(val_sb[:], val_d.ap())
                wsv = wstat[:].rearrange("j (c h o) -> j c h o", c=C,
                                         h=H_LOC)
                wjv = wj_sb[:].rearrange("j (c one o) -> j c one o", c=C,
                                         one=1)
                valv = val_sb[:].rearrange("j (one h two) -> j one h two",
                                           one=1, two=1)
                nc.vector.tensor_mul(
                    wsv, wjv.broadcast_to([pj, C, H_LOC, 32]),
                    valv.broadcast_to([pj, C, H_LOC, 32]))

            with (
                tc.tile_pool(name="gath", bufs=gbufs) as gpool,
                tc.tile_pool(name="osb", bufs=2) as opool,
                tc.tile_pool(name="psum", bufs=4, space="PSUM") as ppool,
            ):
                def emit_iter(it):
                    out_sb = opool.tile([32, H_LOC * L], F32, tag="osb",
                                        name=f"osb_{it}")
                    for hl in range(H_LOC):
                        g = gpool.tile([pj, L * C], F16, tag="g",
                                       name=f"g_{it}_{hl}")
                        nc.gpsimd.indirect_dma_start(
                            out=g[:, :], out_offset=None, in_=x2flat,
                            in_offset=bass.IndirectOffsetOnAxis(
                                ap=idx_sb[:, hl:hl + 1], axis=1))
                        gv = g[:].rearrange("j (l c) -> j l c", c=C)
                        psum = ppool.tile([32, L], F32, tag="ps",
                                          name=f"ps_{it}_{hl}")
                        for c in range(C):
                            stat = wstat[:, (c * H_LOC + hl) * 32:
                                         (c * H_LOC + hl + 1) * 32]
                            nc.tensor.matmul(psum[:], stat, gv[:, :, c],
                                             start=(c == 0),
                                             stop=(c == C - 1))
                        nc.vector.tensor_scalar_add(
                            out_sb[:, hl * L:(hl + 1) * L], psum[:],
                            bias_sb[:])
                    nc.sync.dma_start(
                        out_d.ap().rearrange("o h l -> o (h l)"), out_sb[:])

                if repeat == 1:
                    emit_iter(0)
                else:
                    # Hardware loop: iterations are idempotent (identical
                    # APs), so the body is emitted `unroll` times and
                    # looped on device - program size is independent of
                    # `repeat`. Unrolling amortizes the loop's all-engine
                    # barrier + reset-block drains over several
                    # iterations.
                    assert repeat % unroll == 0, (repeat, unroll)
                    with tc.For_i(0, repeat // unroll):
                        for u in range(unroll):
                            emit_iter(u)
    dedupe_ldweights(nc)
    if split:
        if repeat > 1:
            # The For_i loop emits InstIncSwdgeSem (SWDGE sem rebalance),
            # an InstISA subclass whose .instr bytes are only populated by
            # this pass; without it walrus codegen fails with "ISA wrong
            # length". CoreSim (split=False) interprets the typed form.
            mybir.codegen_inst_isa_subclasses(nc)
        split_multiwaits(nc)
    return nc


_PROGRAM_CACHE = {}


def _get_program(pj, repeat=1):
    key = (pj, repeat)
    if key not in _PROGRAM_CACHE:
        _PROGRAM_CACHE[key] = build_program(pj, repeat)
    return _PROGRAM_CACHE[key]


def kernel(x, psi_vals, weight, bias, psi_ik, psi_lat_out, psi_lat_in,
           psi_lon_in, nlat_out, nlon_out):
    x = np.asarray(x)
    assert x.shape == (B, C, H, L), x.shape
    assert int(nlat_out) == H and int(nlon_out) == L

    in_maps, pj = host_prep(
        x, np.asarray(psi_vals), np.asarray(weight), np.asarray(bias),
        np.asarray(psi_ik), np.asarray(psi_lat_out),
        np.asarray(psi_lat_in), np.asarray(psi_lon_in))

    nc = _get_program(pj)
    res = bass_utils.run_bass_kernel_spmd(nc, in_maps,
                                          core_ids=list(range(N_CORES)))
    out = np.concatenate([res.results[m]["out"] for m in range(N_CORES)],
                         axis=1)
    return out.reshape(B, 32, H, L).astype(np.float32)
